# revision 60
# baseline (speedup 1.0000x reference)
"""Trainium2 Bass kernel for a 2-layer dense-adjacency GAT + attention pooling head.

Contract: kernel(**inputs) takes the FULL (unsharded) numpy inputs and returns the
full output tuple (logits [1,5] f32, Y_hat [1,1] i32, Y_prob [1,5] f32).

Sharding: 1D node partition. Each of the 8 NeuronCores owns 512 of the 4096 nodes:
it computes its rows of each N x N attention matrix and its rows of att @ Wh, with
the per-head node features Wh all-gathered on device between stages. Weights are
replicated.

Math notes:
 - e_ij = leaky_relu(si_i + sj_j) masked by adj; softmax over j. We skip the
   row-max subtraction (softmax is shift-invariant and |si+sj| <= ~10 so exp is
   safe in bf16/f32) and get the softmax denominator for free by appending a
   ones-column to the all-gathered Wh matrix (so one matmul computes both
   numerator att@Wh and denominator).
 - P = exp(leaky(si+sj)) * mask is built in TRANSPOSED layout [j, i] so it feeds
   TensorE directly as the stationary operand (lhsT) with no transposes.
 - Per-head pipeline is split between ScalarE (Prelu->Exp, both in the
   exp_and_others activation-table set so no table reloads) and VectorE
   (exp(si)*exp(sj) rank-1 products + fused scalar_tensor_tensor max) so the
   two elementwise engines run concurrently; TensorE overlaps the att@Wh
   accumulation. Chunks are processed in pairs so Exp and the mask multiply
   run as double-width ops (mask-mul in place), amortizing per-op engine
   overhead. Engine busy is ~balanced (PE/ACT/DVE all ~70-90%).
"""

import numpy as np
import ml_dtypes

NBF = ml_dtypes.bfloat16

N = 4096
NC = 8
ROWS = N // NC            # 512 rows per core
NM = ROWS // 128          # 4 M-tiles per core
NJ = N // 128             # 32 j-chunks
D_IN = 2560
H = 4
D1 = 256
D2 = 64
DENSE = 16
ATT = 6
NCLASS = 5
SLOPE = 0.01

K1 = 21                   # ceil((2560+1)/128) K-chunks for layer-1 Wh matmul
K1P = K1 * 128            # 2688 padded contraction dim
HB1 = D1 + 2              # 258: per-head block [Wh(256) | ones | sj]
WB1 = H * HB1 + H         # 1036: 4 head blocks + 4 si columns
HB2 = D2 + 2              # 66
WB2 = H * HB2 + H         # 268

# head -> elementwise pipeline ("act" = Lrelu+Exp on ScalarE, "dve" = rank-1 exp
# products on VectorE). Paired so each PSUM-sharing pair has one of each.
PIPE = ("act", "dve", "act", "dve")
PAIRS = ((0, 1), (2, 3))

_CACHE = {}


def _build(sim_mode=False):
    import concourse.bacc as bacc
    import concourse.mybir as mybir
    from concourse.tile import TileContext
    from concourse import masks

    BF = mybir.dt.bfloat16
    F32 = mybir.dt.float32
    I32 = mybir.dt.int32
    OP = mybir.AluOpType
    AF = mybir.ActivationFunctionType

    nc = bacc.Bacc("TRN2", target_bir_lowering=False, debug=False, num_devices=NC)

    # ---- dram I/O ----
    hT_d = nc.declare_dram_parameter("hT", [K1P, ROWS], BF, isOutput=False)
    w1_d = nc.declare_dram_parameter("w1e", [K1P, WB1], BF, isOutput=False)
    w2_d = nc.declare_dram_parameter("w2e", [257, WB2], BF, isOutput=False)
    mk_d = nc.declare_dram_parameter("maskT", [N, ROWS], BF, isOutput=False)
    fc1_d = nc.declare_dram_parameter("fc1e", [D2 + 1, DENSE], BF, isOutput=False)
    fc2_d = nc.declare_dram_parameter("fc2e", [32, ATT], BF, isOutput=False)
    fcf_d = nc.declare_dram_parameter("fcfe", [D2 + 1, NCLASS], BF, isOutput=False)

    logit_d = nc.declare_dram_parameter("logits", [1, NCLASS], F32, isOutput=True)
    yhat_d = nc.declare_dram_parameter("yhat", [1, 1], I32, isOutput=True)
    yprob_d = nc.declare_dram_parameter("yprob", [1, NCLASS], F32, isOutput=True)
    if DEBUG_OUT:
        dbg1_d = nc.declare_dram_parameter("dbg1", [ROWS, D1], BF, isOutput=True)
        dbg2_d = nc.declare_dram_parameter("dbg2", [ROWS, D2], BF, isOutput=True)

    ag1_in = nc.dram_tensor("ag1_in", [ROWS, WB1], BF)
    ag1_out = nc.dram_tensor("ag1_out", [N, WB1], BF, addr_space="Shared")
    ag2_in = nc.dram_tensor("ag2_in", [D1, ROWS], BF)
    ag2_out = nc.dram_tensor("ag2_out", [D1 * NC, ROWS], BF, addr_space="Shared")
    ar_in = nc.dram_tensor("ar_in", [D2 + 1, ATT], F32)
    ar_out = nc.dram_tensor("ar_out", [D2 + 1, ATT], F32, addr_space="Shared")

    rg = [list(range(NC))]

    def collective(kind, op, cc_in, cc_out, nrep):
        """Real collective, or (sim_mode) DMA stand-in with similar byte volume."""
        if not sim_mode:
            nc.gpsimd.collective_compute(kind, op, replica_groups=rg,
                                         ins=[cc_in[:, :]], outs=[cc_out[:, :]])
            return
        rows = cc_in.shape[0]
        for b in range(nrep):
            nc.sync.dma_start(out=cc_out[b * rows:(b + 1) * rows, :],
                              in_=cc_in[:, :])

    with TileContext(nc) as tc:
        sb = tc.alloc_tile_pool(name="sb", bufs=1)
        ps = tc.alloc_tile_pool(name="ps", bufs=1, space="PSUM")

        def psum(shape, bank, dtype=F32):
            return ps.tile(shape, dtype, tag=f"b{bank}", name=f"ps{bank}_{nc.next_id()}")

        # ---- constants ----
        identb = sb.tile([128, 128], BF)
        masks.make_identity(nc, identb[:, :])
        identf = sb.tile([128, 128], F32)
        masks.make_identity(nc, identf[:, :])
        ones1 = sb.tile([1, 128], BF)
        nc.vector.memset(ones1, 1.0)
        ones6 = sb.tile([ATT, 1], F32)
        nc.vector.memset(ones6, 1.0)
        onesrow = sb.tile([1, ROWS], BF)
        nc.vector.memset(onesrow, 1.0)

        # ---- resident big tiles ----
        WH1 = sb.tile([128, NJ * WB1], BF)                 # all-gathered layer-1 features
        MT = sb.tile([128, NJ * ROWS], BF)                 # transposed adjacency mask
        # batched mask load: 4 DMAs of 8 chunks each (3D strided)
        MTr = MT.rearrange("p (c w) -> p c w", w=ROWS)
        mkr = mk_d.rearrange("(c p) w -> p c w", p=128)
        for q in range(4):
            nc.sync.dma_start(out=MTr[:, q * 8:(q + 1) * 8, :],
                              in_=mkr[:, q * 8:(q + 1) * 8, :])

        W2s = sb.tile([128, 2 * WB2], BF)
        for k in range(2):
            nc.sync.dma_start(out=W2s[:, k * WB2:(k + 1) * WB2],
                              in_=w2_d[k * 128:(k + 1) * 128, :])
        W2ones = sb.tile([1, WB2], BF)
        nc.sync.dma_start(out=W2ones, in_=w2_d[256:257, :])
        fc1s = sb.tile([D2 + 1, DENSE], BF)
        nc.sync.dma_start(out=fc1s, in_=fc1_d[:, :])
        fc2s = sb.tile([32, ATT], BF)
        nc.sync.dma_start(out=fc2s, in_=fc2_d[:, :])
        fcfs = sb.tile([D2 + 1, NCLASS], BF)
        nc.sync.dma_start(out=fcfs, in_=fcf_d[:, :])

        # ---- phase A: local Wh1 shard = hT.T @ W1ext   [512, 1036] ----
        # hT and W1 resident in SBUF (slots later reused by WH2 / O1T), loaded
        # with a few large strided DMAs; Wh1 staged bf16 then DMA'd to the
        # all-gather input in DRAM.
        # big slot A: W1 (phase A) then O1T (layer 2); disjoint lifetimes
        W1s = sb.tile([128, K1 * WB1], BF, tag="bigA")
        # big slot B: hT (phase A + si1) then WH2 (layer 2)
        hTs = sb.tile([128, K1 * ROWS], BF, tag="bigB")
        hTr = hTs.rearrange("p (k w) -> p k w", w=ROWS)
        hdr = hT_d.rearrange("(k p) w -> p k w", p=128)
        W1r = W1s.rearrange("p (k w) -> p k w", w=WB1)
        wdr = w1_d.rearrange("(k p) w -> p k w", p=128)
        for q in range(3):
            k0, k1 = 7 * q, 7 * (q + 1)
            nc.sync.dma_start(out=hTr[:, k0:k1, :], in_=hdr[:, k0:k1, :])
            nc.sync.dma_start(out=W1r[:, k0:k1, :], in_=wdr[:, k0:k1, :])

        groups1 = [(0, 512), (512, 512), (1024, WB1 - 1024)]
        for g, (g0, gw) in enumerate(groups1):
            accs = [psum([128, gw], (g * NM + m) % 8) for m in range(NM)]
            for k in range(K1):
                for m in range(NM):
                    nc.tensor.matmul(
                        accs[m],
                        lhsT=hTs[:, k * ROWS + m * 128: k * ROWS + (m + 1) * 128],
                        rhs=W1s[:, k * WB1 + g0: k * WB1 + g0 + gw],
                        start=(k == 0), stop=(k == K1 - 1))
            stg = sb.tile([128, NM * gw], BF, name="stg", tag="stg", bufs=1)
            for m in range(NM):
                nc.scalar.copy(stg[:, m * gw:(m + 1) * gw], accs[m])
            nc.sync.dma_start(
                out=ag1_in.rearrange("(m p) w -> p m w", p=128)[:, :, g0:g0 + gw],
                in_=stg.rearrange("p (m w) -> p m w", w=gw))

        # ---- all-gather Wh1 ----
        collective("AllGather", OP.bypass, ag1_in, ag1_out, NC)
        WH1r2 = WH1.rearrange("p (c w) -> p c w", w=WB1)
        agr = ag1_out.rearrange("(c p) w -> p c w", p=128)
        for q in range(8):
            nc.sync.dma_start(out=WH1r2[:, q * 4:(q + 1) * 4, :],
                              in_=agr[:, q * 4:(q + 1) * 4, :])

        # ================= shared attention-layer machinery =================
        def gat_attention(lname, WHT, wbw, hbw, dout, si_mm, out_acc_w):
            """Runs the masked-softmax attention for one GAT layer.

            WHT:  [128, NJ*wbw] all-gathered features (+ones+sj cols per head)
            si_mm(psum_tile): emits matmuls accumulating siT4 [H, 512] into psum
            returns list of 4 sbuf tiles out_m [128, dout] f32 accumulators
            """
            # --- si broadcast tiles + exp precomputes per head ---
            sacc = psum([H, ROWS], 0)
            si_mm(sacc)
            sib4 = sb.tile([H, ROWS], BF, name=f"sib4_{lname}", tag="sib4")
            nc.scalar.copy(sib4, sacc)

            WHr = WHT.rearrange("p (c w) -> p c w", w=wbw)
            sib, ub, u2b, vall, v2all = {}, {}, {}, {}, {}
            for h in range(H):
                # row h of sib4 -> [1, 512] via selector matmul (partition-base
                # rules forbid reading a single row at partition h directly)
                srow_ps = psum([1, ROWS], 5)
                nc.tensor.matmul(srow_ps, lhsT=identb[0:H, h:h + 1], rhs=sib4,
                                 start=True, stop=True)
                srow = sb.tile([1, ROWS], BF, name="srow", tag="srow", bufs=2)
                nc.scalar.copy(srow, srow_ps)
                bc = psum([128, ROWS], 1 + (h % 4))
                nc.tensor.matmul(bc, lhsT=ones1, rhs=srow,
                                 start=True, stop=True)
                sib[h] = sb.tile([128, ROWS], BF, name=f"sib_{lname}_{h}",
                                 tag=f"sibt{h}")
                nc.scalar.copy(sib[h], bc)
                if PIPE[h] == "dve":
                    ub[h] = sb.tile([128, ROWS], BF, name=f"ub_{lname}_{h}",
                                    tag=f"ubt{h}")
                    nc.scalar.activation(ub[h], sib[h], AF.Exp)
                    u2b[h] = sb.tile([128, ROWS], BF, name=f"u2b_{lname}_{h}",
                                     tag=f"u2bt{h}")
                    nc.scalar.activation(u2b[h], sib[h], AF.Exp, scale=SLOPE)
                    sjap = WHr[:, :, h * hbw + dout + 1: h * hbw + dout + 2]
                    vall[h] = sb.tile([128, NJ], F32, name=f"v_{lname}_{h}",
                                      tag=f"vt{h}")
                    v2all[h] = sb.tile([128, NJ], F32, name=f"v2_{lname}_{h}",
                                       tag=f"v2t{h}")
                    # quarter-split so the attention loop only waits for the
                    # first quarter of the (all-gathered / freshly built)
                    # feature tile, not all 32 chunks
                    vr = vall[h].rearrange("p (c o) -> p c o", o=1)
                    v2r = v2all[h].rearrange("p (c o) -> p c o", o=1)
                    for q in range(4):
                        qs = slice(q * 8, (q + 1) * 8)
                        nc.scalar.activation(vr[:, qs, :], sjap[:, qs, :], AF.Exp)
                        nc.scalar.activation(v2r[:, qs, :], sjap[:, qs, :],
                                             AF.Exp, scale=SLOPE)

            # --- attention: P chunks + matmul accumulate ---
            # Chunks are processed in pairs (2c, 2c+1): the two chunks' mask
            # slices are contiguous in MT, so Exp and the mask multiply run as
            # single double-width ops (mask-mul in place) to amortize per-op
            # engine overhead.
            def make_p2(h, cp, mts2):
                pt = sb.tile([128, 2 * ROWS], BF, name="pt", tag=f"pt{h % 2}",
                             bufs=4)
                if PIPE[h] == "act":
                    for s in range(2):
                        c = 2 * cp + s
                        nc.scalar.activation(
                            pt[:, s * ROWS:(s + 1) * ROWS], sib[h], AF.Prelu,
                            bias=WHT[:, c * wbw + h * hbw + dout + 1:
                                     c * wbw + h * hbw + dout + 2],
                            alpha=SLOPE)
                    nc.scalar.activation(pt, pt, AF.Exp)
                else:
                    for s in range(2):
                        c = 2 * cp + s
                        ph = pt[:, s * ROWS:(s + 1) * ROWS]
                        nc.vector.tensor_scalar(
                            ph, ub[h], vall[h][:, c:c + 1], None, op0=OP.mult)
                        nc.vector.scalar_tensor_tensor(
                            ph, u2b[h], v2all[h][:, c:c + 1], ph,
                            op0=OP.mult, op1=OP.max)
                # masked in place, both chunks at once
                nc.vector.tensor_tensor(pt, pt, mts2, op=OP.mult)
                return pt

            def epi(h, m, num_ap, den_ap):
                rden = sb.tile([128, 1], F32, name="rden", tag="rden", bufs=8)
                nc.vector.reciprocal(rden, den_ap)
                if h == 0:
                    nc.scalar.activation(
                        out_acc_w[m], num_ap, AF.Copy, scale=rden)
                else:
                    # out_acc += num * (1/den), fused on VectorE
                    nc.vector.scalar_tensor_tensor(
                        out_acc_w[m], num_ap, rden, out_acc_w[m],
                        op0=OP.mult, op1=OP.add)

            def pair_matmuls(dst_by_h, hs, cp, hb):
                for s in range(2):
                    c = 2 * cp + s
                    for h in hs:
                        pt = dst_by_h[h][1]
                        for m in range(NM):
                            nc.tensor.matmul(
                                dst_by_h[h][0][m],
                                lhsT=pt[:, s * ROWS + m * 128:
                                        s * ROWS + (m + 1) * 128],
                                rhs=WHT[:, c * wbw + h * hbw:
                                        c * wbw + h * hbw + hb],
                                start=(c == 0), stop=(c == NJ - 1))

            hb = dout + 1
            if H * hb * 4 <= 2048:
                # all 4 heads' accumulators fit one PSUM bank per m-tile:
                # single pass over chunk pairs, no pair barrier
                attp = [psum([128, H * hb], m) for m in range(NM)]
                for cp in range(NJ // 2):
                    mts2 = MT[:, 2 * cp * ROWS:(2 * cp + 2) * ROWS]
                    dst = {}
                    for h in range(H):
                        pt = make_p2(h, cp, mts2)
                        dst[h] = ([attp[m][:, h * hb:(h + 1) * hb]
                                   for m in range(NM)], pt)
                    pair_matmuls(dst, range(H), cp, hb)
                for mm in range(NM):
                    out_acc_w.append(sb.tile(
                        [128, dout], BF, name=f"oacc_{lname}{mm}",
                        tag=f"oacc{mm}"))
                for h in range(H):
                    for m in range(NM):
                        epi(h, m, attp[m][:, h * hb: h * hb + dout],
                            attp[m][:, h * hb + dout: (h + 1) * hb])
            else:
                for pi, pair in enumerate(PAIRS):
                    att = {}
                    for j, h in enumerate(pair):
                        att[h] = [psum([128, hb], j * NM + m) for m in range(NM)]
                    for cp in range(NJ // 2):
                        mts2 = MT[:, 2 * cp * ROWS:(2 * cp + 2) * ROWS]
                        dst = {}
                        for h in pair:
                            pt = make_p2(h, cp, mts2)
                            dst[h] = (att[h], pt)
                        pair_matmuls(dst, pair, cp, hb)
                    if pi == 0:
                        for mm in range(NM):
                            out_acc_w.append(sb.tile(
                                [128, dout], BF, name=f"oacc_{lname}{mm}",
                                tag=f"oacc{mm}"))
                    for h in pair:
                        for m in range(NM):
                            epi(h, m, att[h][m][:, 0:dout],
                                att[h][m][:, dout:dout + 1])
            return out_acc_w

        # ---- layer 1 ----
        def si1_mm(sacc):
            for k in range(K1):
                nc.tensor.matmul(
                    sacc,
                    lhsT=W1s[:, k * WB1 + H * HB1: k * WB1 + WB1],
                    rhs=hTs[:, k * ROWS:(k + 1) * ROWS],
                    start=(k == 0), stop=(k == K1 - 1))

        oacc1 = []
        gat_attention("l1", WH1, WB1, HB1, D1, si1_mm, oacc1)

        # tanh(mean over heads) -> bf16, then transpose to [d, i] layout
        o1T = sb.tile([128, 2 * ROWS], BF)     # local out1T: d-chunk k at cols k*512
        for m in range(NM):
            o1m = sb.tile([128, D1], BF, name="o1m", tag="o1m", bufs=2)
            nc.scalar.activation(o1m, oacc1[m], AF.Tanh, scale=1.0 / H)
            if DEBUG_OUT:
                nc.sync.dma_start(out=dbg1_d[m * 128:(m + 1) * 128, :], in_=o1m)
            for k in range(2):
                tp = psum([128, 128], (2 * m + k) % 8, BF)
                nc.tensor.transpose(tp, o1m[:, k * 128:(k + 1) * 128], identb)
                nc.vector.tensor_copy(
                    o1T[:, k * ROWS + m * 128: k * ROWS + (m + 1) * 128], tp)

        # ---- all-gather out1T ----
        for k in range(2):
            nc.sync.dma_start(out=ag2_in[k * 128:(k + 1) * 128, :],
                              in_=o1T[:, k * ROWS:(k + 1) * ROWS])
        collective("AllGather", OP.bypass, ag2_in, ag2_out, NC)
        O1T = sb.tile([128, 2 * NC * ROWS], BF, tag="bigA")  # reuses W1s slot
        O1Tr = O1T.rearrange("p (c w) -> p c w", w=ROWS)
        ag2r = ag2_out.rearrange("(c p) w -> p c w", p=128)
        for q in range(4):
            nc.sync.dma_start(out=O1Tr[:, q * 4:(q + 1) * 4, :],
                              in_=ag2r[:, q * 4:(q + 1) * 4, :])

        # ---- layer-2 features Wh2 = out1ext @ W2ext  (replicated, all 4096 rows) ----
        WH2 = sb.tile([128, NJ * WB2], BF, tag="bigB")       # reuses hTs slot
        for c in range(NJ):
            b, msub = divmod(c, NM)
            acc = psum([128, WB2], c % 8)
            for k in range(2):
                nc.tensor.matmul(
                    acc,
                    lhsT=O1T[:, (2 * b + k) * ROWS + msub * 128:
                             (2 * b + k) * ROWS + (msub + 1) * 128],
                    rhs=W2s[:, k * WB2:(k + 1) * WB2],
                    start=(k == 0), stop=False)
            nc.tensor.matmul(acc, lhsT=ones1, rhs=W2ones, start=False, stop=True)
            if c % 2 == 0:
                nc.scalar.copy(WH2[:, c * WB2:(c + 1) * WB2], acc)
            else:
                nc.vector.tensor_copy(WH2[:, c * WB2:(c + 1) * WB2], acc)

        # ---- layer 2 attention ----
        def si2_mm(sacc):
            for k in range(2):
                nc.tensor.matmul(
                    sacc,
                    lhsT=W2s[:, k * WB2 + H * HB2: k * WB2 + WB2],
                    rhs=o1T[:, k * ROWS:(k + 1) * ROWS],
                    start=(k == 0), stop=(k == 1))

        oacc2 = []
        gat_attention("l2", WH2, WB2, HB2, D2, si2_mm, oacc2)

        # out2 with ones col: [128, 4*65]; and transposed out2T_ext [65, 512]
        out2e = sb.tile([128, NM * (D2 + 1)], BF)
        o2T = sb.tile([D2 + 1, ROWS], BF)
        for m in range(NM):
            c0 = m * (D2 + 1)
            nc.scalar.activation(out2e[:, c0:c0 + D2], oacc2[m], AF.Tanh,
                                 scale=1.0 / H)
            if DEBUG_OUT:
                nc.sync.dma_start(out=dbg2_d[m * 128:(m + 1) * 128, :],
                                  in_=out2e[:, c0:c0 + D2])
            nc.vector.memset(out2e[:, c0 + D2:c0 + D2 + 1], 1.0)
            tp = psum([D2, 128], m % 8, BF)
            nc.tensor.transpose(tp, out2e[:, c0:c0 + D2], identb)
            nc.vector.tensor_copy(o2T[0:D2, m * 128:(m + 1) * 128], tp)
        nc.vector.tensor_copy(o2T[D2:D2 + 1, :], onesrow)

        # ---- pooling head (local rows), then AllReduce ----
        # m-sliced so each 128-node column block flows fc1->tanh->fc2->exp as
        # soon as its out2 transpose lands (overlaps the layer-2 epilogue)
        xTs = sb.tile([32, ROWS], BF)
        nc.vector.memset(xTs, 1.0)
        sps = psum([ATT, ROWS], 1)
        for m in range(NM):
            ms = slice(m * 128, (m + 1) * 128)
            xps = psum([DENSE, 128], 4 + m % 2)
            nc.tensor.matmul(xps, lhsT=fc1s, rhs=o2T[:, ms],
                             start=True, stop=True)
            nc.scalar.activation(xTs[0:DENSE, ms], xps, AF.Tanh)
            nc.tensor.matmul(sps[:, ms], lhsT=fc2s, rhs=xTs[:, ms],
                             start=True, stop=True)
        # packed small-tile arenas (each tiny tile would otherwise burn a
        # 4KB/partition slot)
        tailB = sb.tile([128, 1024], BF)
        tailF = sb.tile([128, 320], F32)
        expsT = tailB[0:ATT, 512:512 + ROWS]
        exps = tailB[:, 8:8 + NM * ATT]
        avgT = tailB[0:D2 + 1, 0:1]
        mt6 = tailF[0:ATT, 0:D2 + 1]
        rd6 = tailF[0:ATT, 72:73]
        emb = tailF[0:ATT, 80:80 + D2]
        msb = tailF[0:D2 + 1, 152:152 + ATT]
        msum = tailF[0:D2 + 1, 160:160 + ATT]
        en = tailF[0:NCLASS, 168:169]
        en1 = tailF[0:NCLASS, 176:177]
        logT = tailF[0:NCLASS, 184:185]
        lrow = tailF[0:1, 192:192 + NCLASS]
        e5 = tailF[0:1, 200:200 + NCLASS]
        s5 = tailF[0:1, 208:209]
        r5 = tailF[0:1, 216:217]
        yp = tailF[0:1, 224:224 + NCLASS]
        mx = tailF[0:1, 232:233]
        ge = tailF[0:1, 240:240 + NCLASS]
        pen = tailF[0:1, 248:248 + NCLASS]
        iotf = tailF[0:1, 256:256 + NCLASS]
        idx = tailF[0:1, 264:264 + NCLASS]
        am = tailF[0:1, 272:273]

        nc.scalar.activation(expsT, sps, AF.Exp)
        for m in range(NM):
            tp = psum([128, ATT], 2 + m % 4, BF)
            nc.tensor.transpose(tp, expsT[:, m * 128:(m + 1) * 128],
                                identb[0:ATT, 0:ATT])
            nc.vector.tensor_copy(exps[:, m * ATT:(m + 1) * ATT], tp)
        mps = psum([D2 + 1, ATT], 6)
        for m in range(NM):
            nc.tensor.matmul(mps, lhsT=out2e[:, m * (D2 + 1):(m + 1) * (D2 + 1)],
                             rhs=exps[:, m * ATT:(m + 1) * ATT],
                             start=(m == 0), stop=(m == NM - 1))
        nc.vector.tensor_copy(msb, mps)
        nc.sync.dma_start(out=ar_in[:, :], in_=msb)
        collective("AllReduce", OP.add, ar_in, ar_out, 1)
        nc.sync.dma_start(out=msum, in_=ar_out[:, :])

        # emb = (M / den).T ; avg = mean over heads ; logits = sigmoid(avg@fcf+b)
        mtp = psum([ATT, D2 + 1], 7)
        nc.tensor.transpose(mtp, msum, identf[0:D2 + 1, 0:D2 + 1])
        nc.vector.tensor_copy(mt6, mtp)
        nc.vector.reciprocal(rd6, mt6[:, D2:D2 + 1])
        nc.vector.tensor_scalar(emb, mt6[:, 0:D2], rd6, 1.0 / ATT,
                                op0=OP.mult, op1=OP.mult)
        aps = psum([D2, 1], 0)
        nc.tensor.matmul(aps, lhsT=emb, rhs=ones6, start=True, stop=True)
        nc.scalar.copy(avgT[0:D2, :], aps)
        nc.vector.tensor_copy(avgT[D2:D2 + 1, :], onesrow[:, 0:1])
        lps = psum([NCLASS, 1], 1)
        nc.tensor.matmul(lps, lhsT=fcfs, rhs=avgT, start=True, stop=True)
        # sigmoid via exp set: 1/(1+exp(-x))
        nc.scalar.activation(en, lps, AF.Exp, scale=-1.0)
        nc.vector.tensor_scalar(en1, en, 1.0, None, op0=OP.add)
        nc.vector.reciprocal(logT, en1)
        ltp = psum([1, NCLASS], 2)
        nc.tensor.transpose(ltp, logT, identf[0:NCLASS, 0:NCLASS])
        nc.vector.tensor_copy(lrow, ltp)
        nc.sync.dma_start(out=logit_d[:, :], in_=lrow)

        # Y_prob = softmax(logits)
        nc.scalar.activation(e5, lrow, AF.Exp)
        nc.vector.reduce_sum(s5, e5, axis=mybir.AxisListType.X)
        nc.vector.reciprocal(r5, s5)
        nc.vector.tensor_scalar(yp, e5, r5, None, op0=OP.mult)
        nc.sync.dma_start(out=yprob_d[:, :], in_=yp)

        # Y_hat = argmax(logits) (first max index)
        nc.vector.reduce_max(mx, lrow, axis=mybir.AxisListType.X)
        nc.vector.tensor_scalar(ge, lrow, mx, None, op0=OP.is_ge)
        nc.vector.tensor_scalar(pen, ge, -1e9, 1e9, op0=OP.mult, op1=OP.add)
        iot = sb.tile([1, NCLASS], I32)
        nc.gpsimd.iota(iot, pattern=[[1, NCLASS]], base=0, channel_multiplier=0)
        nc.vector.tensor_copy(iotf, iot)
        nc.vector.tensor_tensor(idx, pen, iotf, op=OP.add)
        nc.vector.tensor_reduce(am, idx, axis=mybir.AxisListType.X, op=OP.min)
        ami = sb.tile([1, 1], I32)
        nc.vector.tensor_copy(ami, am)
        nc.sync.dma_start(out=yhat_d[:, :], in_=ami)

        ps.release()
        sb.release()

    nc.compile()
    return nc


def _prep_inputs(inputs):
    h = np.asarray(inputs["h"], np.float32)
    adj = np.asarray(inputs["adj"])
    W1 = np.asarray(inputs["W1"], np.float32)
    a1i = np.asarray(inputs["a1i"], np.float32)
    a1j = np.asarray(inputs["a1j"], np.float32)
    W2 = np.asarray(inputs["W2"], np.float32)
    a2i = np.asarray(inputs["a2i"], np.float32)
    a2j = np.asarray(inputs["a2j"], np.float32)

    w1e = np.zeros((K1P, WB1), np.float32)
    for hh in range(H):
        w1e[:D_IN, hh * HB1: hh * HB1 + D1] = W1[hh]
        w1e[D_IN, hh * HB1 + D1] = 1.0
        w1e[:D_IN, hh * HB1 + D1 + 1] = W1[hh] @ a1j[hh]
        w1e[:D_IN, H * HB1 + hh] = W1[hh] @ a1i[hh]
    w1e = w1e.astype(NBF)

    w2e = np.zeros((257, WB2), np.float32)
    for hh in range(H):
        w2e[:D1, hh * HB2: hh * HB2 + D2] = W2[hh]
        w2e[D1, hh * HB2 + D2] = 1.0
        w2e[:D1, hh * HB2 + D2 + 1] = W2[hh] @ a2j[hh]
        w2e[:D1, H * HB2 + hh] = W2[hh] @ a2i[hh]
    w2e = w2e.astype(NBF)

    fc1e = np.concatenate([np.asarray(inputs["fc1_w"], np.float32),
                           np.asarray(inputs["fc1_b"], np.float32)[None, :]],
                          axis=0).astype(NBF)
    fc2e = np.zeros((32, ATT), np.float32)
    fc2e[:DENSE] = np.asarray(inputs["fc2_w"], np.float32)
    fc2e[DENSE] = np.asarray(inputs["fc2_b"], np.float32)
    fc2e = fc2e.astype(NBF)
    fcfe = np.concatenate([np.asarray(inputs["fcf_w"], np.float32),
                           np.asarray(inputs["fcf_b"], np.float32)[None, :]],
                          axis=0).astype(NBF)

    maskf = (adj > 0).astype(np.float32)

    in_maps = []
    for r in range(NC):
        rows = slice(r * ROWS, (r + 1) * ROWS)
        hT = np.zeros((K1P, ROWS), np.float32)
        hT[:D_IN] = h[rows].T
        hT[D_IN] = 1.0
        in_maps.append({
            "hT": hT.astype(NBF),
            "w1e": w1e,
            "w2e": w2e,
            "maskT": np.ascontiguousarray(maskf[rows].T).astype(NBF),
            "fc1e": fc1e,
            "fc2e": fc2e,
            "fcfe": fcfe,
        })
    return in_maps


def _get_nc():
    if "nc" not in _CACHE:
        _CACHE["nc"] = _build()
    return _CACHE["nc"]


def kernel(**inputs):
    from concourse.bass_utils import run_bass_kernel_spmd

    nc = _get_nc()
    in_maps = _prep_inputs(inputs)
    try:
        res = run_bass_kernel_spmd(nc, in_maps, core_ids=list(range(NC)))
    except Exception:
        # transient NRT_EXEC_UNIT_UNRECOVERABLE has been observed on first
        # touch of freshly-claimed devices; one retry clears it
        res = run_bass_kernel_spmd(nc, in_maps, core_ids=list(range(NC)))
    _CACHE["last_res"] = res
    R = res.results[0]
    logits = np.asarray(R["logits"], np.float32).reshape(1, NCLASS)
    yhat = np.asarray(R["yhat"], np.int32).reshape(1, 1)
    yprob = np.asarray(R["yprob"], np.float32).reshape(1, NCLASS)
    return (logits, yhat, yprob)


# revision 61
# speedup vs baseline: 1.0006x; 1.0006x over previous
"""Trainium2 Bass kernel for a 2-layer dense-adjacency GAT + attention pooling head.

Contract: kernel(**inputs) takes the FULL (unsharded) numpy inputs and returns the
full output tuple (logits [1,5] f32, Y_hat [1,1] i32, Y_prob [1,5] f32).

Sharding: 1D node partition. Each of the 8 NeuronCores owns 512 of the 4096 nodes:
it computes its rows of each N x N attention matrix and its rows of att @ Wh, with
the per-head node features Wh all-gathered on device between stages. Weights are
replicated.

Math notes:
 - e_ij = leaky_relu(si_i + sj_j) masked by adj; softmax over j. We skip the
   row-max subtraction (softmax is shift-invariant and |si+sj| <= ~10 so exp is
   safe in bf16/f32) and get the softmax denominator for free by appending a
   ones-column to the all-gathered Wh matrix (so one matmul computes both
   numerator att@Wh and denominator).
 - P = exp(leaky(si+sj)) * mask is built in TRANSPOSED layout [j, i] so it feeds
   TensorE directly as the stationary operand (lhsT) with no transposes.
 - Per-head pipeline is split between ScalarE (Prelu->Exp, both in the
   exp_and_others activation-table set so no table reloads) and VectorE
   (exp(si)*exp(sj) rank-1 products + fused scalar_tensor_tensor max) so the
   two elementwise engines run concurrently; TensorE overlaps the att@Wh
   accumulation. Chunks are processed in pairs so Exp and the mask multiply
   run as double-width ops (mask-mul in place), amortizing per-op engine
   overhead. Engine busy is ~balanced (PE/ACT/DVE all ~70-90%).
"""

import numpy as np
import ml_dtypes

NBF = ml_dtypes.bfloat16

N = 4096
NC = 8
ROWS = N // NC            # 512 rows per core
NM = ROWS // 128          # 4 M-tiles per core
NJ = N // 128             # 32 j-chunks
D_IN = 2560
H = 4
D1 = 256
D2 = 64
DENSE = 16
ATT = 6
NCLASS = 5
SLOPE = 0.01

K1 = 21                   # ceil((2560+1)/128) K-chunks for layer-1 Wh matmul
K1P = K1 * 128            # 2688 padded contraction dim
HB1 = D1 + 2              # 258: per-head block [Wh(256) | ones | sj]
WB1 = H * HB1 + H         # 1036: 4 head blocks + 4 si columns
HB2 = D2 + 2              # 66
WB2 = H * HB2 + H         # 268

# head -> elementwise pipeline ("act" = Lrelu+Exp on ScalarE, "dve" = rank-1 exp
# products on VectorE). Paired so each PSUM-sharing pair has one of each.
PIPE = ("act", "dve", "act", "dve")
PAIRS = ((0, 1), (2, 3))

_CACHE = {}


def _build(sim_mode=False):
    import concourse.bacc as bacc
    import concourse.mybir as mybir
    from concourse.tile import TileContext
    from concourse import masks

    BF = mybir.dt.bfloat16
    F32 = mybir.dt.float32
    I32 = mybir.dt.int32
    OP = mybir.AluOpType
    AF = mybir.ActivationFunctionType

    nc = bacc.Bacc("TRN2", target_bir_lowering=False, debug=False, num_devices=NC)

    # ---- dram I/O ----
    hT_d = nc.declare_dram_parameter("hT", [K1P, ROWS], BF, isOutput=False)
    w1_d = nc.declare_dram_parameter("w1e", [K1P, WB1], BF, isOutput=False)
    w2_d = nc.declare_dram_parameter("w2e", [257, WB2], BF, isOutput=False)
    mk_d = nc.declare_dram_parameter("maskT", [N, ROWS], BF, isOutput=False)
    fc1_d = nc.declare_dram_parameter("fc1e", [D2 + 1, DENSE], BF, isOutput=False)
    fc2_d = nc.declare_dram_parameter("fc2e", [32, ATT], BF, isOutput=False)
    fcf_d = nc.declare_dram_parameter("fcfe", [D2 + 1, NCLASS], BF, isOutput=False)

    logit_d = nc.declare_dram_parameter("logits", [1, NCLASS], F32, isOutput=True)
    yhat_d = nc.declare_dram_parameter("yhat", [1, 1], I32, isOutput=True)
    yprob_d = nc.declare_dram_parameter("yprob", [1, NCLASS], F32, isOutput=True)
    if DEBUG_OUT:
        dbg1_d = nc.declare_dram_parameter("dbg1", [ROWS, D1], BF, isOutput=True)
        dbg2_d = nc.declare_dram_parameter("dbg2", [ROWS, D2], BF, isOutput=True)

    ag1_in = nc.dram_tensor("ag1_in", [ROWS, WB1], BF)
    ag1_out = nc.dram_tensor("ag1_out", [N, WB1], BF, addr_space="Shared")
    ag2_in = nc.dram_tensor("ag2_in", [D1, ROWS], BF)
    ag2_out = nc.dram_tensor("ag2_out", [D1 * NC, ROWS], BF, addr_space="Shared")
    ar_in = nc.dram_tensor("ar_in", [D2 + 1, ATT], F32)
    ar_out = nc.dram_tensor("ar_out", [D2 + 1, ATT], F32, addr_space="Shared")

    rg = [list(range(NC))]

    def collective(kind, op, cc_in, cc_out, nrep):
        """Real collective, or (sim_mode) DMA stand-in with similar byte volume."""
        if not sim_mode:
            nc.gpsimd.collective_compute(kind, op, replica_groups=rg,
                                         ins=[cc_in[:, :]], outs=[cc_out[:, :]])
            return
        rows = cc_in.shape[0]
        for b in range(nrep):
            nc.sync.dma_start(out=cc_out[b * rows:(b + 1) * rows, :],
                              in_=cc_in[:, :])

    with TileContext(nc) as tc:
        sb = tc.alloc_tile_pool(name="sb", bufs=1)
        ps = tc.alloc_tile_pool(name="ps", bufs=1, space="PSUM")

        def psum(shape, bank, dtype=F32):
            return ps.tile(shape, dtype, tag=f"b{bank}", name=f"ps{bank}_{nc.next_id()}")

        # ---- constants ----
        identb = sb.tile([128, 128], BF)
        masks.make_identity(nc, identb[:, :])
        identf = sb.tile([128, 128], F32)
        masks.make_identity(nc, identf[:, :])
        ones1 = sb.tile([1, 128], BF)
        nc.vector.memset(ones1, 1.0)
        ones6 = sb.tile([ATT, 1], F32)
        nc.vector.memset(ones6, 1.0)
        onesrow = sb.tile([1, ROWS], BF)
        nc.vector.memset(onesrow, 1.0)

        # ---- resident big tiles ----
        WH1 = sb.tile([128, NJ * WB1], BF)                 # all-gathered layer-1 features
        MT = sb.tile([128, NJ * ROWS], BF)                 # transposed adjacency mask
        # batched mask load: 4 DMAs of 8 chunks each (3D strided)
        MTr = MT.rearrange("p (c w) -> p c w", w=ROWS)
        mkr = mk_d.rearrange("(c p) w -> p c w", p=128)
        for q in range(4):
            nc.sync.dma_start(out=MTr[:, q * 8:(q + 1) * 8, :],
                              in_=mkr[:, q * 8:(q + 1) * 8, :])

        W2s = sb.tile([128, 2 * WB2], BF)
        for k in range(2):
            nc.sync.dma_start(out=W2s[:, k * WB2:(k + 1) * WB2],
                              in_=w2_d[k * 128:(k + 1) * 128, :])
        W2ones = sb.tile([1, WB2], BF)
        nc.sync.dma_start(out=W2ones, in_=w2_d[256:257, :])
        fc1s = sb.tile([D2 + 1, DENSE], BF)
        nc.sync.dma_start(out=fc1s, in_=fc1_d[:, :])
        fc2s = sb.tile([32, ATT], BF)
        nc.sync.dma_start(out=fc2s, in_=fc2_d[:, :])
        fcfs = sb.tile([D2 + 1, NCLASS], BF)
        nc.sync.dma_start(out=fcfs, in_=fcf_d[:, :])

        # ---- phase A: local Wh1 shard = hT.T @ W1ext   [512, 1036] ----
        # hT and W1 resident in SBUF (slots later reused by WH2 / O1T), loaded
        # with a few large strided DMAs; Wh1 staged bf16 then DMA'd to the
        # all-gather input in DRAM.
        # big slot A: W1 (phase A) then O1T (layer 2); disjoint lifetimes
        W1s = sb.tile([128, K1 * WB1], BF, tag="bigA")
        # big slot B: hT (phase A + si1) then WH2 (layer 2)
        hTs = sb.tile([128, K1 * ROWS], BF, tag="bigB")
        hTr = hTs.rearrange("p (k w) -> p k w", w=ROWS)
        hdr = hT_d.rearrange("(k p) w -> p k w", p=128)
        W1r = W1s.rearrange("p (k w) -> p k w", w=WB1)
        wdr = w1_d.rearrange("(k p) w -> p k w", p=128)
        for q in range(3):
            k0, k1 = 7 * q, 7 * (q + 1)
            nc.sync.dma_start(out=hTr[:, k0:k1, :], in_=hdr[:, k0:k1, :])
            nc.sync.dma_start(out=W1r[:, k0:k1, :], in_=wdr[:, k0:k1, :])

        groups1 = [(0, 512), (512, 512), (1024, WB1 - 1024)]
        for g, (g0, gw) in enumerate(groups1):
            accs = [psum([128, gw], (g * NM + m) % 8) for m in range(NM)]
            for k in range(K1):
                for m in range(NM):
                    nc.tensor.matmul(
                        accs[m],
                        lhsT=hTs[:, k * ROWS + m * 128: k * ROWS + (m + 1) * 128],
                        rhs=W1s[:, k * WB1 + g0: k * WB1 + g0 + gw],
                        start=(k == 0), stop=(k == K1 - 1))
            stg = sb.tile([128, NM * gw], BF, name="stg", tag="stg", bufs=1)
            for m in range(NM):
                nc.scalar.copy(stg[:, m * gw:(m + 1) * gw], accs[m])
            nc.sync.dma_start(
                out=ag1_in.rearrange("(m p) w -> p m w", p=128)[:, :, g0:g0 + gw],
                in_=stg.rearrange("p (m w) -> p m w", w=gw))

        # ---- all-gather Wh1 ----
        collective("AllGather", OP.bypass, ag1_in, ag1_out, NC)
        WH1r2 = WH1.rearrange("p (c w) -> p c w", w=WB1)
        agr = ag1_out.rearrange("(c p) w -> p c w", p=128)
        for q in range(8):
            nc.sync.dma_start(out=WH1r2[:, q * 4:(q + 1) * 4, :],
                              in_=agr[:, q * 4:(q + 1) * 4, :])

        # ================= shared attention-layer machinery =================
        def gat_attention(lname, WHT, wbw, hbw, dout, si_mm, out_acc_w):
            """Runs the masked-softmax attention for one GAT layer.

            WHT:  [128, NJ*wbw] all-gathered features (+ones+sj cols per head)
            si_mm(psum_tile): emits matmuls accumulating siT4 [H, 512] into psum
            returns list of 4 sbuf tiles out_m [128, dout] f32 accumulators
            """
            # --- si broadcast tiles + exp precomputes per head ---
            sacc = psum([H, ROWS], 0)
            si_mm(sacc)
            sib4 = sb.tile([H, ROWS], BF, name=f"sib4_{lname}", tag="sib4")
            nc.scalar.copy(sib4, sacc)

            WHr = WHT.rearrange("p (c w) -> p c w", w=wbw)
            sib, ub, u2b, vall, v2all = {}, {}, {}, {}, {}
            for h in range(H):
                # row h of sib4 -> [1, 512] via selector matmul (partition-base
                # rules forbid reading a single row at partition h directly)
                srow_ps = psum([1, ROWS], 5)
                nc.tensor.matmul(srow_ps, lhsT=identb[0:H, h:h + 1], rhs=sib4,
                                 start=True, stop=True)
                srow = sb.tile([1, ROWS], BF, name="srow", tag="srow", bufs=2)
                nc.scalar.copy(srow, srow_ps)
                bc = psum([128, ROWS], 1 + (h % 4))
                nc.tensor.matmul(bc, lhsT=ones1, rhs=srow,
                                 start=True, stop=True)
                sib[h] = sb.tile([128, ROWS], BF, name=f"sib_{lname}_{h}",
                                 tag=f"sibt{h}")
                nc.scalar.copy(sib[h], bc)
                if PIPE[h] == "dve":
                    ub[h] = sb.tile([128, ROWS], BF, name=f"ub_{lname}_{h}",
                                    tag=f"ubt{h}")
                    nc.scalar.activation(ub[h], sib[h], AF.Exp)
                    u2b[h] = sb.tile([128, ROWS], BF, name=f"u2b_{lname}_{h}",
                                     tag=f"u2bt{h}")
                    nc.scalar.activation(u2b[h], sib[h], AF.Exp, scale=SLOPE)
                    sjap = WHr[:, :, h * hbw + dout + 1: h * hbw + dout + 2]
                    vall[h] = sb.tile([128, NJ], F32, name=f"v_{lname}_{h}",
                                      tag=f"vt{h}")
                    v2all[h] = sb.tile([128, NJ], F32, name=f"v2_{lname}_{h}",
                                       tag=f"v2t{h}")
                    # quarter-split so the attention loop only waits for the
                    # first quarter of the (all-gathered / freshly built)
                    # feature tile, not all 32 chunks
                    vr = vall[h].rearrange("p (c o) -> p c o", o=1)
                    v2r = v2all[h].rearrange("p (c o) -> p c o", o=1)
                    for q in range(4):
                        qs = slice(q * 8, (q + 1) * 8)
                        nc.scalar.activation(vr[:, qs, :], sjap[:, qs, :], AF.Exp)
                        nc.scalar.activation(v2r[:, qs, :], sjap[:, qs, :],
                                             AF.Exp, scale=SLOPE)

            # --- attention: P chunks + matmul accumulate ---
            # Chunks are processed in pairs (2c, 2c+1): the two chunks' mask
            # slices are contiguous in MT, so Exp and the mask multiply run as
            # single double-width ops (mask-mul in place) to amortize per-op
            # engine overhead.
            def make_p2(h, cp, mts2):
                pt = sb.tile([128, 2 * ROWS], BF, name="pt", tag=f"pt{h % 2}",
                             bufs=4)
                if PIPE[h] == "act":
                    for s in range(2):
                        c = 2 * cp + s
                        nc.scalar.activation(
                            pt[:, s * ROWS:(s + 1) * ROWS], sib[h], AF.Prelu,
                            bias=WHT[:, c * wbw + h * hbw + dout + 1:
                                     c * wbw + h * hbw + dout + 2],
                            alpha=SLOPE)
                    nc.scalar.activation(pt, pt, AF.Exp)
                else:
                    for s in range(2):
                        c = 2 * cp + s
                        ph = pt[:, s * ROWS:(s + 1) * ROWS]
                        nc.vector.tensor_scalar(
                            ph, ub[h], vall[h][:, c:c + 1], None, op0=OP.mult)
                        nc.vector.scalar_tensor_tensor(
                            ph, u2b[h], v2all[h][:, c:c + 1], ph,
                            op0=OP.mult, op1=OP.max)
                # masked in place, both chunks at once
                nc.vector.tensor_tensor(pt, pt, mts2, op=OP.mult)
                return pt

            def epi(h, m, num_ap, rden):
                if h == 0:
                    nc.scalar.activation(
                        out_acc_w[m], num_ap, AF.Copy, scale=rden)
                else:
                    # out_acc += num * (1/den), fused on VectorE
                    nc.vector.scalar_tensor_tensor(
                        out_acc_w[m], num_ap, rden, out_acc_w[m],
                        op0=OP.mult, op1=OP.add)

            def pair_matmuls(dst_by_h, hs, cp, hb):
                for s in range(2):
                    c = 2 * cp + s
                    for h in hs:
                        pt = dst_by_h[h][1]
                        for m in range(NM):
                            nc.tensor.matmul(
                                dst_by_h[h][0][m],
                                lhsT=pt[:, s * ROWS + m * 128:
                                        s * ROWS + (m + 1) * 128],
                                rhs=WHT[:, c * wbw + h * hbw:
                                        c * wbw + h * hbw + hb],
                                start=(c == 0), stop=(c == NJ - 1))

            hb = dout + 1
            if H * hb * 4 <= 2048:
                # all 4 heads' accumulators fit one PSUM bank per m-tile:
                # single pass over chunk pairs, no pair barrier
                attp = [psum([128, H * hb], m) for m in range(NM)]
                for cp in range(NJ // 2):
                    mts2 = MT[:, 2 * cp * ROWS:(2 * cp + 2) * ROWS]
                    dst = {}
                    for h in range(H):
                        pt = make_p2(h, cp, mts2)
                        dst[h] = ([attp[m][:, h * hb:(h + 1) * hb]
                                   for m in range(NM)], pt)
                    pair_matmuls(dst, range(H), cp, hb)
                for mm in range(NM):
                    out_acc_w.append(sb.tile(
                        [128, dout], BF, name=f"oacc_{lname}{mm}",
                        tag=f"oacc{mm}"))
                for m in range(NM):
                    # all 4 heads' denominators in one strided reciprocal
                    rden4 = sb.tile([128, H], F32, name="rden4", tag="rden",
                                    bufs=8)
                    nc.vector.reciprocal(
                        rden4.rearrange("p (h o) -> p h o", o=1),
                        attp[m].rearrange("p (h w) -> p h w",
                                          w=hb)[:, :, dout:dout + 1])
                    for h in range(H):
                        epi(h, m, attp[m][:, h * hb: h * hb + dout],
                            rden4[:, h:h + 1])
            else:
                for pi, pair in enumerate(PAIRS):
                    att = {}
                    for j, h in enumerate(pair):
                        att[h] = [psum([128, hb], j * NM + m) for m in range(NM)]
                    for cp in range(NJ // 2):
                        mts2 = MT[:, 2 * cp * ROWS:(2 * cp + 2) * ROWS]
                        dst = {}
                        for h in pair:
                            pt = make_p2(h, cp, mts2)
                            dst[h] = (att[h], pt)
                        pair_matmuls(dst, pair, cp, hb)
                    if pi == 0:
                        for mm in range(NM):
                            out_acc_w.append(sb.tile(
                                [128, dout], BF, name=f"oacc_{lname}{mm}",
                                tag=f"oacc{mm}"))
                    for h in pair:
                        for m in range(NM):
                            rden = sb.tile([128, 1], F32, name="rden",
                                           tag="rden", bufs=8)
                            nc.vector.reciprocal(rden,
                                                 att[h][m][:, dout:dout + 1])
                            epi(h, m, att[h][m][:, 0:dout], rden)
            return out_acc_w

        # ---- layer 1 ----
        def si1_mm(sacc):
            for k in range(K1):
                nc.tensor.matmul(
                    sacc,
                    lhsT=W1s[:, k * WB1 + H * HB1: k * WB1 + WB1],
                    rhs=hTs[:, k * ROWS:(k + 1) * ROWS],
                    start=(k == 0), stop=(k == K1 - 1))

        oacc1 = []
        gat_attention("l1", WH1, WB1, HB1, D1, si1_mm, oacc1)

        # tanh(mean over heads) -> bf16, then transpose to [d, i] layout
        o1T = sb.tile([128, 2 * ROWS], BF)     # local out1T: d-chunk k at cols k*512
        for m in range(NM):
            o1m = sb.tile([128, D1], BF, name="o1m", tag="o1m", bufs=2)
            nc.scalar.activation(o1m, oacc1[m], AF.Tanh, scale=1.0 / H)
            if DEBUG_OUT:
                nc.sync.dma_start(out=dbg1_d[m * 128:(m + 1) * 128, :], in_=o1m)
            for k in range(2):
                tp = psum([128, 128], (2 * m + k) % 8, BF)
                nc.tensor.transpose(tp, o1m[:, k * 128:(k + 1) * 128], identb)
                nc.vector.tensor_copy(
                    o1T[:, k * ROWS + m * 128: k * ROWS + (m + 1) * 128], tp)

        # ---- all-gather out1T ----
        for k in range(2):
            nc.sync.dma_start(out=ag2_in[k * 128:(k + 1) * 128, :],
                              in_=o1T[:, k * ROWS:(k + 1) * ROWS])
        collective("AllGather", OP.bypass, ag2_in, ag2_out, NC)
        O1T = sb.tile([128, 2 * NC * ROWS], BF, tag="bigA")  # reuses W1s slot
        O1Tr = O1T.rearrange("p (c w) -> p c w", w=ROWS)
        ag2r = ag2_out.rearrange("(c p) w -> p c w", p=128)
        for q in range(4):
            nc.sync.dma_start(out=O1Tr[:, q * 4:(q + 1) * 4, :],
                              in_=ag2r[:, q * 4:(q + 1) * 4, :])

        # ---- layer-2 features Wh2 = out1ext @ W2ext  (replicated, all 4096 rows) ----
        WH2 = sb.tile([128, NJ * WB2], BF, tag="bigB")       # reuses hTs slot
        for c in range(NJ):
            b, msub = divmod(c, NM)
            acc = psum([128, WB2], c % 8)
            for k in range(2):
                nc.tensor.matmul(
                    acc,
                    lhsT=O1T[:, (2 * b + k) * ROWS + msub * 128:
                             (2 * b + k) * ROWS + (msub + 1) * 128],
                    rhs=W2s[:, k * WB2:(k + 1) * WB2],
                    start=(k == 0), stop=False)
            nc.tensor.matmul(acc, lhsT=ones1, rhs=W2ones, start=False, stop=True)
            if c % 2 == 0:
                nc.scalar.copy(WH2[:, c * WB2:(c + 1) * WB2], acc)
            else:
                nc.vector.tensor_copy(WH2[:, c * WB2:(c + 1) * WB2], acc)

        # ---- layer 2 attention ----
        def si2_mm(sacc):
            for k in range(2):
                nc.tensor.matmul(
                    sacc,
                    lhsT=W2s[:, k * WB2 + H * HB2: k * WB2 + WB2],
                    rhs=o1T[:, k * ROWS:(k + 1) * ROWS],
                    start=(k == 0), stop=(k == 1))

        oacc2 = []
        gat_attention("l2", WH2, WB2, HB2, D2, si2_mm, oacc2)

        # out2 with ones col: [128, 4*65]; and transposed out2T_ext [65, 512]
        out2e = sb.tile([128, NM * (D2 + 1)], BF)
        o2T = sb.tile([D2 + 1, ROWS], BF)
        for m in range(NM):
            c0 = m * (D2 + 1)
            nc.scalar.activation(out2e[:, c0:c0 + D2], oacc2[m], AF.Tanh,
                                 scale=1.0 / H)
            if DEBUG_OUT:
                nc.sync.dma_start(out=dbg2_d[m * 128:(m + 1) * 128, :],
                                  in_=out2e[:, c0:c0 + D2])
            nc.vector.memset(out2e[:, c0 + D2:c0 + D2 + 1], 1.0)
            tp = psum([D2, 128], m % 8, BF)
            nc.tensor.transpose(tp, out2e[:, c0:c0 + D2], identb)
            nc.vector.tensor_copy(o2T[0:D2, m * 128:(m + 1) * 128], tp)
        nc.vector.tensor_copy(o2T[D2:D2 + 1, :], onesrow)

        # ---- pooling head (local rows), then AllReduce ----
        # m-sliced so each 128-node column block flows fc1->tanh->fc2->exp as
        # soon as its out2 transpose lands (overlaps the layer-2 epilogue)
        xTs = sb.tile([32, ROWS], BF)
        nc.vector.memset(xTs, 1.0)
        sps = psum([ATT, ROWS], 1)
        for m in range(NM):
            ms = slice(m * 128, (m + 1) * 128)
            xps = psum([DENSE, 128], 4 + m % 2)
            nc.tensor.matmul(xps, lhsT=fc1s, rhs=o2T[:, ms],
                             start=True, stop=True)
            nc.scalar.activation(xTs[0:DENSE, ms], xps, AF.Tanh)
            nc.tensor.matmul(sps[:, ms], lhsT=fc2s, rhs=xTs[:, ms],
                             start=True, stop=True)
        # packed small-tile arenas (each tiny tile would otherwise burn a
        # 4KB/partition slot)
        tailB = sb.tile([128, 1024], BF)
        tailF = sb.tile([128, 320], F32)
        expsT = tailB[0:ATT, 512:512 + ROWS]
        exps = tailB[:, 8:8 + NM * ATT]
        avgT = tailB[0:D2 + 1, 0:1]
        mt6 = tailF[0:ATT, 0:D2 + 1]
        rd6 = tailF[0:ATT, 72:73]
        emb = tailF[0:ATT, 80:80 + D2]
        msb = tailF[0:D2 + 1, 152:152 + ATT]
        msum = tailF[0:D2 + 1, 160:160 + ATT]
        en = tailF[0:NCLASS, 168:169]
        en1 = tailF[0:NCLASS, 176:177]
        logT = tailF[0:NCLASS, 184:185]
        lrow = tailF[0:1, 192:192 + NCLASS]
        e5 = tailF[0:1, 200:200 + NCLASS]
        s5 = tailF[0:1, 208:209]
        r5 = tailF[0:1, 216:217]
        yp = tailF[0:1, 224:224 + NCLASS]
        mx = tailF[0:1, 232:233]
        ge = tailF[0:1, 240:240 + NCLASS]
        pen = tailF[0:1, 248:248 + NCLASS]
        iotf = tailF[0:1, 256:256 + NCLASS]
        idx = tailF[0:1, 264:264 + NCLASS]
        am = tailF[0:1, 272:273]

        nc.scalar.activation(expsT, sps, AF.Exp)
        for m in range(NM):
            tp = psum([128, ATT], 2 + m % 4, BF)
            nc.tensor.transpose(tp, expsT[:, m * 128:(m + 1) * 128],
                                identb[0:ATT, 0:ATT])
            nc.vector.tensor_copy(exps[:, m * ATT:(m + 1) * ATT], tp)
        mps = psum([D2 + 1, ATT], 6)
        for m in range(NM):
            nc.tensor.matmul(mps, lhsT=out2e[:, m * (D2 + 1):(m + 1) * (D2 + 1)],
                             rhs=exps[:, m * ATT:(m + 1) * ATT],
                             start=(m == 0), stop=(m == NM - 1))
        nc.vector.tensor_copy(msb, mps)
        nc.sync.dma_start(out=ar_in[:, :], in_=msb)
        collective("AllReduce", OP.add, ar_in, ar_out, 1)
        nc.sync.dma_start(out=msum, in_=ar_out[:, :])

        # emb = (M / den).T ; avg = mean over heads ; logits = sigmoid(avg@fcf+b)
        mtp = psum([ATT, D2 + 1], 7)
        nc.tensor.transpose(mtp, msum, identf[0:D2 + 1, 0:D2 + 1])
        nc.vector.tensor_copy(mt6, mtp)
        nc.vector.reciprocal(rd6, mt6[:, D2:D2 + 1])
        nc.vector.tensor_scalar(emb, mt6[:, 0:D2], rd6, 1.0 / ATT,
                                op0=OP.mult, op1=OP.mult)
        aps = psum([D2, 1], 0)
        nc.tensor.matmul(aps, lhsT=emb, rhs=ones6, start=True, stop=True)
        nc.scalar.copy(avgT[0:D2, :], aps)
        nc.vector.tensor_copy(avgT[D2:D2 + 1, :], onesrow[:, 0:1])
        lps = psum([NCLASS, 1], 1)
        nc.tensor.matmul(lps, lhsT=fcfs, rhs=avgT, start=True, stop=True)
        # sigmoid via exp set: 1/(1+exp(-x))
        nc.scalar.activation(en, lps, AF.Exp, scale=-1.0)
        nc.vector.tensor_scalar(en1, en, 1.0, None, op0=OP.add)
        nc.vector.reciprocal(logT, en1)
        ltp = psum([1, NCLASS], 2)
        nc.tensor.transpose(ltp, logT, identf[0:NCLASS, 0:NCLASS])
        nc.vector.tensor_copy(lrow, ltp)
        nc.sync.dma_start(out=logit_d[:, :], in_=lrow)

        # Y_prob = softmax(logits)
        nc.scalar.activation(e5, lrow, AF.Exp)
        nc.vector.reduce_sum(s5, e5, axis=mybir.AxisListType.X)
        nc.vector.reciprocal(r5, s5)
        nc.vector.tensor_scalar(yp, e5, r5, None, op0=OP.mult)
        nc.sync.dma_start(out=yprob_d[:, :], in_=yp)

        # Y_hat = argmax(logits) (first max index)
        nc.vector.reduce_max(mx, lrow, axis=mybir.AxisListType.X)
        nc.vector.tensor_scalar(ge, lrow, mx, None, op0=OP.is_ge)
        nc.vector.tensor_scalar(pen, ge, -1e9, 1e9, op0=OP.mult, op1=OP.add)
        iot = sb.tile([1, NCLASS], I32)
        nc.gpsimd.iota(iot, pattern=[[1, NCLASS]], base=0, channel_multiplier=0)
        nc.vector.tensor_copy(iotf, iot)
        nc.vector.tensor_tensor(idx, pen, iotf, op=OP.add)
        nc.vector.tensor_reduce(am, idx, axis=mybir.AxisListType.X, op=OP.min)
        ami = sb.tile([1, 1], I32)
        nc.vector.tensor_copy(ami, am)
        nc.sync.dma_start(out=yhat_d[:, :], in_=ami)

        ps.release()
        sb.release()

    nc.compile()
    return nc


def _prep_inputs(inputs):
    h = np.asarray(inputs["h"], np.float32)
    adj = np.asarray(inputs["adj"])
    W1 = np.asarray(inputs["W1"], np.float32)
    a1i = np.asarray(inputs["a1i"], np.float32)
    a1j = np.asarray(inputs["a1j"], np.float32)
    W2 = np.asarray(inputs["W2"], np.float32)
    a2i = np.asarray(inputs["a2i"], np.float32)
    a2j = np.asarray(inputs["a2j"], np.float32)

    w1e = np.zeros((K1P, WB1), np.float32)
    for hh in range(H):
        w1e[:D_IN, hh * HB1: hh * HB1 + D1] = W1[hh]
        w1e[D_IN, hh * HB1 + D1] = 1.0
        w1e[:D_IN, hh * HB1 + D1 + 1] = W1[hh] @ a1j[hh]
        w1e[:D_IN, H * HB1 + hh] = W1[hh] @ a1i[hh]
    w1e = w1e.astype(NBF)

    w2e = np.zeros((257, WB2), np.float32)
    for hh in range(H):
        w2e[:D1, hh * HB2: hh * HB2 + D2] = W2[hh]
        w2e[D1, hh * HB2 + D2] = 1.0
        w2e[:D1, hh * HB2 + D2 + 1] = W2[hh] @ a2j[hh]
        w2e[:D1, H * HB2 + hh] = W2[hh] @ a2i[hh]
    w2e = w2e.astype(NBF)

    fc1e = np.concatenate([np.asarray(inputs["fc1_w"], np.float32),
                           np.asarray(inputs["fc1_b"], np.float32)[None, :]],
                          axis=0).astype(NBF)
    fc2e = np.zeros((32, ATT), np.float32)
    fc2e[:DENSE] = np.asarray(inputs["fc2_w"], np.float32)
    fc2e[DENSE] = np.asarray(inputs["fc2_b"], np.float32)
    fc2e = fc2e.astype(NBF)
    fcfe = np.concatenate([np.asarray(inputs["fcf_w"], np.float32),
                           np.asarray(inputs["fcf_b"], np.float32)[None, :]],
                          axis=0).astype(NBF)

    maskf = (adj > 0).astype(np.float32)

    in_maps = []
    for r in range(NC):
        rows = slice(r * ROWS, (r + 1) * ROWS)
        hT = np.zeros((K1P, ROWS), np.float32)
        hT[:D_IN] = h[rows].T
        hT[D_IN] = 1.0
        in_maps.append({
            "hT": hT.astype(NBF),
            "w1e": w1e,
            "w2e": w2e,
            "maskT": np.ascontiguousarray(maskf[rows].T).astype(NBF),
            "fc1e": fc1e,
            "fc2e": fc2e,
            "fcfe": fcfe,
        })
    return in_maps


def _get_nc():
    if "nc" not in _CACHE:
        _CACHE["nc"] = _build()
    return _CACHE["nc"]


def kernel(**inputs):
    from concourse.bass_utils import run_bass_kernel_spmd

    nc = _get_nc()
    in_maps = _prep_inputs(inputs)
    try:
        res = run_bass_kernel_spmd(nc, in_maps, core_ids=list(range(NC)))
    except Exception:
        # transient NRT_EXEC_UNIT_UNRECOVERABLE has been observed on first
        # touch of freshly-claimed devices; one retry clears it
        res = run_bass_kernel_spmd(nc, in_maps, core_ids=list(range(NC)))
    _CACHE["last_res"] = res
    R = res.results[0]
    logits = np.asarray(R["logits"], np.float32).reshape(1, NCLASS)
    yhat = np.asarray(R["yhat"], np.int32).reshape(1, 1)
    yprob = np.asarray(R["yprob"], np.float32).reshape(1, NCLASS)
    return (logits, yhat, yprob)
